# revision 1
# baseline (speedup 1.0000x reference)
"""Causal self-attention (B=2, T=2048, C=1024, 16 heads) on 8 trn2 cores.

Sharding: 2 batches x 4 head-groups (4 heads each). Per core, t-chunk-major
pipeline: stream x columns per 512-wide chunk, project qkv for the chunk,
then run causal attention for the chunk immediately (causality: chunk c only
attends to k/v tiles <= c). Scores stay transposed [tk, tq]; y^T accumulates
in psum with a ones-column denominator row; normalization via K=1 broadcast
matmul + fast reciprocal. y^T is AllGathered across the 4 cores of each
batch per chunk (overlapped with the next chunk's attention), followed by a
transposed column-slice output projection per chunk. Host re-transposes and
concatenates the 8 [256, T] slices.
"""
import numpy as np
import ml_dtypes

import concourse.bacc as bacc
import concourse.mybir as mybir
import concourse.tile as tile
from concourse.bass_utils import run_bass_kernel_spmd

F32 = mybir.dt.float32
F32R = mybir.dt.float32r
BF16 = mybir.dt.bfloat16
EXP = mybir.ActivationFunctionType.Exp

B, T, C = 2, 2048, 1024
NH, HD = 16, 64
NCORES = 8
NG = 4            # head groups (tensor-parallel within a batch)
GC = 256          # features per group (4 heads * 64)
NFT = C // 128    # 8 feature tiles
NTT = T // 128    # 16 t tiles
NCH = T // 512    # 4 tq chunks

_nc_cache = {}


def build_nc():
    nc = bacc.Bacc("TRN2", target_bir_lowering=False, debug=False, num_devices=NCORES)
    xT = nc.dram_tensor("xT", [C, T], F32R, kind="ExternalInput")
    wq = nc.dram_tensor("wq", [C, GC], F32R, kind="ExternalInput")
    wk = nc.dram_tensor("wk", [C, GC], F32R, kind="ExternalInput")
    wv = nc.dram_tensor("wv", [C, GC], F32R, kind="ExternalInput")
    wpr = nc.dram_tensor("wpr", [GC, C], F32R, kind="ExternalInput")
    tri = nc.dram_tensor("tri", [128, 128], F32R, kind="ExternalInput")
    ones = nc.dram_tensor("ones", [128, 64], F32R, kind="ExternalInput")
    outP = nc.dram_tensor("outP", [C, T], F32, kind="ExternalOutput")

    with tile.TileContext(nc) as tc:
        with (
            tc.tile_pool(name="xc", bufs=1) as xcp,        # streamed x chunks
            tc.tile_pool(name="wpool", bufs=1) as wpool,
            tc.tile_pool(name="qk", bufs=1) as qkpool,
            tc.tile_pool(name="vpool", bufs=1) as vpool,
            tc.tile_pool(name="work", bufs=1) as work,
            tc.tile_pool(name="ytpool", bufs=1) as ytpool,
            tc.tile_pool(name="dram", bufs=1, space="DRAM") as dram,
            tc.tile_pool(name="psum", bufs=1, space="PSUM") as ps,
        ):
            # ---------------- loads: tiny consts, then x chunk0 + wq interleaved ----------------
            tri_sb = wpool.tile([128, 128], F32R, name="tri_sb")
            nc.sync.dma_start(tri_sb[:], tri[:])
            ones_sb = wpool.tile([128, 64], F32R, name="ones_sb")
            nc.sync.dma_start(ones_sb[:], ones[:])

            xc_tiles = {}

            def load_xc(c):
                xc = [
                    xcp.tile([128, 512], F32R, tag="xc", bufs=16, name=f"xc{c}_{i}")
                    for i in range(NFT)
                ]
                for i in range(NFT):
                    nc.sync.dma_start(
                        xc[i][:], xT[128 * i : 128 * (i + 1), 512 * c : 512 * (c + 1)]
                    )
                    if c == 0:
                        nc.sync.dma_start(wqt[i][:], wq[128 * i : 128 * (i + 1), :])
                xc_tiles[c] = xc

            wqt = [wpool.tile([128, GC], F32R, tag="wq", bufs=8, name=f"wqt{i}") for i in range(NFT)]
            wkt = [wpool.tile([128, GC], F32R, tag="wk", bufs=8, name=f"wkt{i}") for i in range(NFT)]
            wvt = [wpool.tile([128, GC], F32R, tag="wv", bufs=8, name=f"wvt{i}") for i in range(NFT)]
            wprt = [wpool.tile([128, C], F32R, tag="wpr", bufs=2, name=f"wprt{f}") for f in range(2)]

            load_xc(0)
            for i in range(NFT):
                nc.sync.dma_start(wkt[i][:], wk[128 * i : 128 * (i + 1), :])
            for i in range(NFT):
                nc.sync.dma_start(wvt[i][:], wv[128 * i : 128 * (i + 1), :])
            for f in range(2):
                nc.sync.dma_start(wprt[f][:], wpr[128 * f : 128 * (f + 1), :])

            # persistent per-core tensors
            qT = [qkpool.tile([128, T], F32R, tag="qT", bufs=2, name=f"qT{p}") for p in range(2)]
            kT = [qkpool.tile([128, T], F32R, tag="kT", bufs=2, name=f"kT{p}") for p in range(2)]
            vb = [vpool.tile([128, 260], F32R, tag="v", bufs=NTT, name=f"vb{tt}") for tt in range(NTT)]
            yT_sb = [
                ytpool.tile([64, T], F32R, tag="yt", bufs=4, name=f"yTsb{ph}")
                for ph in range(4)
            ]
            def proj_partial(c):
                # pack the 4 heads' normalized y^T for chunk c into [128, 512]
                # tiles (cross-partition move => DMA), then project against the
                # row-slice of W_proj; host sums partials across the 4 cores.
                ypk = [
                    work.tile([128, 512], F32R, tag="ypk", bufs=4, name=f"ypk{c}_{f}")
                    for f in range(2)
                ]
                for pp in range(2):
                    for h in range(2):
                        nc.sync.dma_start(
                            ypk[pp][64 * h : 64 * (h + 1), :],
                            yT_sb[2 * pp + h][:, 512 * c : 512 * (c + 1)],
                        )
                for u in range(NFT):
                    opp = ps.tile([128, 512], F32, tag="mix", bufs=2, name=f"opp{c}{u}")
                    for f in range(2):
                        nc.tensor.matmul(
                            opp[:],
                            wprt[f][:, 128 * u : 128 * (u + 1)],
                            ypk[f][:],
                            start=(f == 0),
                            stop=(f == 1),
                        )
                    osb = work.tile([128, 512], F32, tag="osb", bufs=3, name=f"osb{c}{u}")
                    nc.vector.tensor_copy(osb[:], opp[:])
                    nc.sync.dma_start(
                        outP[128 * u : 128 * (u + 1), 512 * c : 512 * (c + 1)], osb[:]
                    )

            pending_norm = []

            def flush_norms():
                for (p_, c_, h_, yrw) in pending_norm:
                    bc = ps.tile([64, 512], F32, tag="mix", bufs=2, name=f"bc{p_}{c_}{h_}")
                    nc.tensor.matmul(
                        bc[:], ones_sb[64:65, :], yrw[64:65, :], start=True, stop=True
                    )
                    rcp = work.tile([64, 512], F32, tag="rcp", bufs=2, name=f"rcp{p_}{c_}{h_}")
                    nc.vector.reciprocal_approx_fast(rcp[:], bc[:])
                    nc.vector.tensor_mul(
                        yT_sb[2 * p_ + h_][:, 512 * c_ : 512 * (c_ + 1)],
                        yrw[0:64, :],
                        rcp[:],
                    )
                pending_norm.clear()

            # ---------------- per-chunk pipeline ----------------
            for c in range(NCH):
                xc = xc_tiles[c]
                # qkv for this chunk
                for p in range(2):
                    qps = ps.tile([128, 512], F32, tag="mix", bufs=2, name=f"qps{p}_{c}")
                    for i in range(NFT):
                        nc.tensor.matmul(
                            qps[:],
                            wqt[i][:, 128 * p : 128 * (p + 1)],
                            xc[i][:],
                            start=(i == 0),
                            stop=(i == NFT - 1),
                        )
                    nc.vector.tensor_copy(qT[p][:, 512 * c : 512 * (c + 1)], qps[:])
                    kps = ps.tile([128, 512], F32, tag="mix", bufs=2, name=f"kps{p}_{c}")
                    for i in range(NFT):
                        nc.tensor.matmul(
                            kps[:],
                            wkt[i][:, 128 * p : 128 * (p + 1)],
                            xc[i][:],
                            start=(i == 0),
                            stop=(i == NFT - 1),
                        )
                    nc.vector.tensor_copy(kT[p][:, 512 * c : 512 * (c + 1)], kps[:])
                for ttl in range(4):
                    tt = 4 * c + ttl
                    vps = ps.tile([128, 256], F32, tag="mix", bufs=2, name=f"vps{tt}")
                    for i in range(NFT):
                        nc.tensor.matmul(
                            vps[:],
                            xc[i][:, 128 * ttl : 128 * (ttl + 1)],
                            wvt[i][:],
                            start=(i == 0),
                            stop=(i == NFT - 1),
                        )
                    nc.sync.dma_start(
                        vb[tt][:].rearrange("p (s c) -> p s c", s=4)[:, :, 64:65],
                        ones_sb[:, 0:4].rearrange("p (s o) -> p s o", o=1),
                    )
                    nc.vector.tensor_copy(
                        vb[tt][:].rearrange("p (s c) -> p s c", s=4)[:, :, 0:64],
                        vps[:].rearrange("p (s c) -> p s c", s=4),
                    )
                # prefetch next chunk's x right behind this chunk's compute wave
                if c + 1 < NCH:
                    load_xc(c + 1)

                flush_norms()
                if c >= 1:
                    proj_partial(c - 1)

                # attention for this chunk; j-loop software-pipelined one
                # stage so PE runs scores(j+1) while ACT computes exp(j)
                for p in range(2):
                    yta = [
                        ps.tile([65, 512], F32, tag=f"yta{h}", bufs=1, name=f"yta{p}{c}{h}")
                        for h in range(2)
                    ]

                    def scores_exp(j):
                        d = j - 4 * c
                        off = 128 * max(d, 0)
                        sps = ps.tile([128, 1024], F32, tag="sps", bufs=2, name=f"sps{p}{c}{j}")
                        for h in range(2):
                            nc.tensor.matmul(
                                sps[:, 512 * h + off : 512 * (h + 1)],
                                kT[p][64 * h : 64 * (h + 1), 128 * j : 128 * (j + 1)],
                                qT[p][64 * h : 64 * (h + 1), 512 * c + off : 512 * (c + 1)],
                                start=True,
                                stop=True,
                            )
                        es = work.tile([128, 1024], F32R, tag="es", bufs=4, name=f"es{p}{c}{j}")
                        nc.scalar.activation(
                            es[:].rearrange("p (g n) -> p g n", g=2)[:, :, off:512],
                            sps[:].rearrange("p (g n) -> p g n", g=2)[:, :, off:512],
                            EXP,
                            scale=0.125,
                        )
                        if d >= 0:
                            for h in range(2):
                                nc.vector.tensor_mul(
                                    es[:, 512 * h + off : 512 * h + off + 128],
                                    es[:, 512 * h + off : 512 * h + off + 128],
                                    tri_sb[:],
                                )
                        return es

                    def av(j, es):
                        d = j - 4 * c
                        off = 128 * max(d, 0)
                        for h in range(2):
                            hs = 2 * p + h
                            nc.tensor.matmul(
                                yta[h][:, off:512],
                                vb[j][:, 65 * hs : 65 * hs + 65],
                                es[:, 512 * h + off : 512 * (h + 1)],
                                start=(j == 0),
                                stop=(j == 4 * c + 3),
                            )

                    prev = None
                    for j in range(4 * c + 4):
                        es = scores_exp(j)
                        if prev is not None:
                            av(*prev)
                        prev = (j, es)
                    av(*prev)
                    # evacuate yta now (releases psum); defer the normalize
                    # (bc matmul + recip + mul) so PE is not stalled here
                    for h in range(2):
                        yrw = work.tile([65, 512], F32R, tag="yrw", bufs=6, name=f"yrw{p}{c}{h}")
                        nc.vector.tensor_copy(yrw[:], yta[h][:])
                        pending_norm.append((p, c, h, yrw))

            flush_norms()
            proj_partial(NCH - 1)

    nc.compile()
    return nc


def _get_nc():
    if "nc" not in _nc_cache:
        _nc_cache["nc"] = build_nc()
    return _nc_cache["nc"]


def _in_maps(x, W_attn, W_proj):
    tri = np.triu(np.ones((128, 128), np.float32))
    ones = np.ones((128, 64), np.float32)
    maps = []
    for core in range(NCORES):
        b, g = core // NG, core % NG
        lo = g * GC
        maps.append(
            {
                "xT": np.ascontiguousarray(x[b].T),
                "wq": np.ascontiguousarray(W_attn[:, lo : lo + GC]),
                "wk": np.ascontiguousarray(W_attn[:, C + lo : C + lo + GC]),
                "wv": np.ascontiguousarray(W_attn[:, 2 * C + lo : 2 * C + lo + GC]),
                "wpr": np.ascontiguousarray(W_proj[lo : lo + GC, :]),
                "tri": tri,
                "ones": ones,
            }
        )
    return maps


def kernel(x, W_attn, W_proj, **run_kwargs):
    x = np.asarray(x, np.float32)
    W_attn = np.asarray(W_attn, np.float32)
    W_proj = np.asarray(W_proj, np.float32)
    nc = _get_nc()
    res = run_bass_kernel_spmd(
        nc, _in_maps(x, W_attn, W_proj), core_ids=list(range(NCORES)), **run_kwargs
    )
    out = np.empty((B, T, C), np.float32)
    for b in range(B):
        acc = res.results[NG * b]["outP"].copy()
        for g in range(1, NG):
            acc += res.results[NG * b + g]["outP"]
        out[b] = acc.T
    if run_kwargs:
        kernel.last_result = res
    return out



# revision 25
# speedup vs baseline: 1.2705x; 1.2705x over previous
"""Causal self-attention (B=2, T=2048, C=1024, 16 heads) on 8 trn2 cores.

Sharding: 2 batches x 4 head-groups (4 heads each per core). Per core,
t-chunk-major pipeline in bf16 (fp32 psum accumulation):
  - stream x columns per 512-wide chunk (one DMA per chunk), project q/k into
    a shared [128, 2, T] bf16 tile per head-pair and v into a [128, 16, 4, 65]
    bf16 tile with a ones-column for the softmax denominator;
  - scores stay transposed [tk, tq] in a dedicated psum ring, exp on ACT into
    bf16 es tiles, causal diagonal masked via a tri multiply on DVE; the
    j-loop is software-pipelined (scores j+1 issued before av j);
  - AV is recast as out[tq, d+1] psum with the es 128x128 column block as the
    matmul stationary operand and v (65 cols) moving, so each matmul streams
    only 65 rows; the denominator lands per-partition and is normalized with
    reciprocal + broadcast multiply on DVE;
  - normalized y blocks are PE-transposed into a packed [128, 2, 512] bf16
    psum tile at chunk end (replacing pack DMAs), copied once to SBUF, and
    consumed by the output projection; qkv of chunk c+1 and the projection of
    chunk c-1 are emitted as fine-grained thunks interleaved into the
    attention j-loop so the PE always has independent work while ACT computes
    exponentials; projection partials are copied psum->sbuf on GPSIMD and
    DMAed out; the host sums the 4 per-core partials per batch.
"""
import numpy as np
import ml_dtypes

import concourse.bacc as bacc
import concourse.mybir as mybir
import concourse.tile as tile
from concourse.bass_utils import run_bass_kernel_spmd

F32 = mybir.dt.float32
BF16 = mybir.dt.bfloat16
EXP = mybir.ActivationFunctionType.Exp

B, T, C = 2, 2048, 1024
NH, HD = 16, 64
NCORES = 8
NG = 4            # head groups (tensor-parallel within a batch)
GC = 256          # features per group (4 heads * 64)
NFT = C // 128    # 8 feature tiles
NTT = T // 128    # 16 t tiles
NCH = T // 512    # 4 tq chunks

_nc_cache = {}


def build_nc():
    nc = bacc.Bacc("TRN2", target_bir_lowering=False, debug=False, num_devices=NCORES)
    xT = nc.dram_tensor("xT", [C, T], BF16, kind="ExternalInput")
    wq = nc.dram_tensor("wq", [C, GC], BF16, kind="ExternalInput")
    wk = nc.dram_tensor("wk", [C, GC], BF16, kind="ExternalInput")
    wv = nc.dram_tensor("wv", [C, GC], BF16, kind="ExternalInput")
    wpr = nc.dram_tensor("wpr", [GC, C], BF16, kind="ExternalInput")
    tri = nc.dram_tensor("tri", [128, 128], BF16, kind="ExternalInput")
    ident = nc.dram_tensor("ident", [128, 128], BF16, kind="ExternalInput")
    outP = nc.dram_tensor("outP", [C, T], F32, kind="ExternalOutput")

    with tile.TileContext(nc) as tc:
        with (
            tc.tile_pool(name="consts", bufs=1) as consts,
            tc.tile_pool(name="xp", bufs=1) as xp,
            tc.tile_pool(name="wp", bufs=1) as wp,
            tc.tile_pool(name="qk", bufs=1) as qkp,
            tc.tile_pool(name="vbp", bufs=1) as vbp,
            tc.tile_pool(name="work", bufs=1) as work,
            tc.tile_pool(name="psum", bufs=1, space="PSUM") as ps,
        ):
            # weight tiles: partition = contraction (c) dim
            wqt = wp.tile([128, NFT, GC], BF16, name="wqt")
            wkt = wp.tile([128, NFT, GC], BF16, name="wkt")
            wvt = wp.tile([128, NFT, GC], BF16, name="wvt")
            wprt = wp.tile([128, 2, C], BF16, name="wprt")

            wq_r = wq[:].rearrange("(i p) f -> p i f", p=128)
            wk_r = wk[:].rearrange("(i p) f -> p i f", p=128)
            wv_r = wv[:].rearrange("(i p) f -> p i f", p=128)
            xT_r = xT[:].rearrange("(i p) t -> p i t", p=128)

            xct = {}

            def load_xc(c, split=False):
                xc = xp.tile([128, NFT, 512], BF16, tag="xc", bufs=2, name=f"xc{c}")
                if split:
                    # interleave x / wq feature slices so the first q matmuls
                    # start as early as possible (finest at the front)
                    for lo, hi in ((0, 1), (1, 2), (2, 4), (4, 6), (6, 8)):
                        nc.sync.dma_start(
                            xc[:, lo:hi, :], xT_r[:, lo:hi, 512 * c : 512 * (c + 1)]
                        )
                        nc.sync.dma_start(wqt[:, lo:hi, :], wq_r[:, lo:hi, :])
                else:
                    nc.sync.dma_start(xc[:], xT_r[:, :, 512 * c : 512 * (c + 1)])
                xct[c] = xc

            load_xc(0, split=True)
            nc.sync.dma_start(wvt[:], wv_r)
            nc.sync.dma_start(wkt[:], wk_r)
            nc.sync.dma_start(wprt[:], wpr[:].rearrange("(f d) c -> d f c", d=128))
            tri_sb = consts.tile([128, 128], BF16, name="tri_sb")
            nc.sync.dma_start(tri_sb[:], tri[:])
            id_sb = consts.tile([128, 128], BF16, name="id_sb")
            nc.sync.dma_start(id_sb[:], ident[:])

            # persistent per-core tensors
            qkT = [qkp.tile([128, 2, T], BF16, name=f"qkT{p}") for p in range(2)]
            # v with ones column at index 64 per (ttile, head-slot)
            VB = vbp.tile([128, NTT, 4, 65], BF16, name="VB")
            nc.vector.memset(VB[:, :, :, 64:65], 1.0)
            # packed normalized y^T per chunk (one per chunk: projections are
            # deferred into late-chunk windows where PE otherwise idles)
            ypk_sb = [
                work.tile([128, 2, 512], BF16, tag="ypk", bufs=NCH, name=f"ypk{m}")
                for m in range(NCH)
            ]

            def qk_half(c, p, which):
                # one [128, 512] projection (q or k) for head-pair p, chunk c
                w = wqt if which == 0 else wkt
                qkps = ps.tile([128, 512], F32, tag="big", bufs=2, name=f"qk{c}{p}{which}")
                for i in range(NFT):
                    nc.tensor.matmul(
                        qkps[:],
                        w[:, i, 128 * p : 128 * (p + 1)],
                        xct[c][:, i, :],
                        start=(i == 0),
                        stop=(i == NFT - 1),
                    )
                nc.vector.tensor_copy(qkT[p][:, which, 512 * c : 512 * (c + 1)], qkps[:])

            def v_pair(c, tl2):
                # v for two 128-t tiles (tl2, tl2+1) of chunk c.
                # one accumulation group for the whole psum bank: start only on
                # the very first write, stop on the last (psum start lazily
                # zeroes the entire 2KB bank region)
                vps = ps.tile([128, 512], F32, tag="big", bufs=2, name=f"v{c}{tl2}")
                for tl in (tl2, tl2 + 1):
                    for i in range(NFT):
                        nc.tensor.matmul(
                            vps[:, 256 * (tl - tl2) : 256 * (tl - tl2 + 1)],
                            xct[c][:, i, 128 * tl : 128 * (tl + 1)],
                            wvt[:, i, :],
                            start=(tl == tl2 and i == 0),
                            stop=(tl == tl2 + 1 and i == NFT - 1),
                        )
                nc.vector.tensor_copy(
                    VB[:, 4 * c + tl2 : 4 * c + tl2 + 2, :, 0:64],
                    vps[:].rearrange("p (s h d) -> p s h d", s=2, h=4),
                )

            def proj_u(c, u, eng=None):
                # one 128-row output slice of the projection for chunk c
                yp = ypk_sb[c]
                opp = ps.tile([128, 512], F32, tag="big", bufs=2, name=f"opp{c}{u}")
                for p in range(2):
                    nc.tensor.matmul(
                        opp[:],
                        wprt[:, p, 128 * u : 128 * (u + 1)],
                        yp[:, p, :],
                        start=(p == 0),
                        stop=(p == 1),
                    )
                ob = work.tile([128, 512], F32, tag="osb", bufs=6, name=f"ob{c}{u}")
                if eng == "s":
                    nc.scalar.copy(ob[:], opp[:])
                else:
                    nc.vector.tensor_copy(ob[:], opp[:])
                nc.sync.dma_start(
                    outP[:]
                    .rearrange("(u d) t -> d u t", d=128)[
                        :, u, 512 * c : 512 * (c + 1)
                    ],
                    ob[:],
                )

            def qkv_thunks(c, skip_qk0=False):
                pairs = [(p, w) for p in range(2) for w in range(2)]
                if skip_qk0:
                    pairs = pairs[2:]
                return (
                    [lambda tl2=tl2: v_pair(c, tl2) for tl2 in (0, 2)]
                    + [lambda p=p, w=w: qk_half(c, p, w) for p, w in pairs]
                )

            def proj_thunks(c):
                return [lambda u=u: proj_u(c, u) for u in range(NFT)]

            def attention(c, thunks, last=False):
                nj = 4 * c + 4
                npts = 2 * nj
                emitted = 0
                pt = 0

                def maybe_thunk():
                    nonlocal emitted, pt
                    while emitted < len(thunks) and emitted <= pt * len(thunks) // npts:
                        thunks[emitted]()
                        emitted += 1
                    pt += 1

                if last:
                    ypkps = ps.tile(
                        [128, 2, 512], BF16, tag="big", bufs=2, name=f"ypkps{c}"
                    )
                def emit_transpose(pp, h, yn, q4):
                    # psum zero-region state is per partition: one accumulation
                    # group per 64-partition half of the ypkps bank (start on
                    # its first transpose, stop on its last); every write hits
                    # fresh (lazily zeroed) bytes so values pass through
                    nc.tensor.matmul(
                        ypkps[64 * h : 64 * (h + 1), pp, 128 * q4 : 128 * (q4 + 1)],
                        yn[:, q4, :],
                        id_sb[:],
                        start=(pp == 0 and q4 == 0),
                        stop=(pp == 1 and q4 == 3),
                        is_transpose=True,
                        # the interpreter's group-check bookkeeping mishandles
                        # 64-partition-offset psum APs; execution semantics
                        # (lazy zero + first-touch overwrite) remain checked
                        skip_group_check=True,
                    )

                yns = []
                for p in range(2):
                    yq = [
                        ps.tile([128, 260], F32, tag="yq", bufs=2, name=f"yq{c}{p}{h}")
                        for h in range(2)
                    ]

                    def scores_exp(j):
                        d = j - 4 * c
                        off = 128 * max(d, 0)
                        sps = ps.tile(
                            [128, 1024], F32, tag="sps", bufs=2, name=f"sps{c}{p}{j}"
                        )
                        for h in range(2):
                            nc.tensor.matmul(
                                sps[:, 512 * h + off : 512 * (h + 1)],
                                qkT[p][64 * h : 64 * (h + 1), 1, 128 * j : 128 * (j + 1)],
                                qkT[p][64 * h : 64 * (h + 1), 0, 512 * c + off : 512 * (c + 1)],
                                start=True,
                                stop=True,
                            )
                        es = work.tile([128, 1024], BF16, tag="es", bufs=4, name=f"es{c}{p}{j}")
                        nc.scalar.activation(
                            es[:].rearrange("p (g n) -> p g n", g=2)[:, :, off:512],
                            sps[:].rearrange("p (g n) -> p g n", g=2)[:, :, off:512],
                            EXP,
                            scale=0.125,
                        )
                        if d >= 0:
                            for h in range(2):
                                nc.vector.tensor_mul(
                                    es[:, 512 * h + off : 512 * h + off + 128],
                                    es[:, 512 * h + off : 512 * h + off + 128],
                                    tri_sb[:],
                                )
                        return es

                    def av(j, es):
                        # one accumulation group per yq bank: start only on the
                        # first write (j=0, q4=0 — its lazy zero covers the
                        # whole bank), stop on the very last
                        d = j - 4 * c
                        for h in range(2):
                            hs = 2 * p + h
                            for q4 in range(max(d, 0), 4):
                                nc.tensor.matmul(
                                    yq[h][:, 65 * q4 : 65 * (q4 + 1)],
                                    es[:, 512 * h + 128 * q4 : 512 * h + 128 * (q4 + 1)],
                                    VB[:, j, hs, :],
                                    start=(j == 0 and q4 == 0),
                                    stop=(j == 4 * c + 3 and q4 == 3),
                                )

                    prev = None
                    for j in range(nj):
                        es = scores_exp(j)
                        maybe_thunk()
                        if prev is not None:
                            av(*prev)
                        prev = (j, es)
                        maybe_thunk()
                    av(*prev)

                    for h in range(2):
                        rd = work.tile([128, 4], F32, tag="rd", bufs=2, name=f"rd{c}{p}{h}")
                        nc.vector.reciprocal_approx_fast(
                            rd[:], yq[h][:].rearrange("p (s n) -> p s n", s=4)[:, :, 64]
                        )
                        yn = work.tile(
                            [128, 4, 64], BF16, tag="yn", bufs=4, name=f"yn{c}{p}{h}"
                        )
                        nc.vector.tensor_mul(
                            yn[:],
                            yq[h][:].rearrange("p (s n) -> p s n", s=4)[:, :, 0:64],
                            rd[:, :, None].broadcast_to([128, 4, 64]),
                        )
                        if last:
                            # transpose + pack immediately so the final
                            # projection's tail is as short as possible
                            for q4 in range(4):
                                emit_transpose(p, h, yn, q4)
                        else:
                            yns.append((p, h, yn))

                    if last:
                        nc.vector.tensor_copy(
                            ypk_sb[c][:, p : p + 1, :], ypkps[:, p : p + 1, :]
                        )

                # drain any leftover thunks before the transpose burst
                while emitted < len(thunks):
                    thunks[emitted]()
                    emitted += 1

                if not last:
                    ypkps = ps.tile(
                        [128, 2, 512], BF16, tag="big", bufs=2, name=f"ypkps{c}"
                    )
                    for (p, h, yn) in yns:
                        for q4 in range(4):
                            emit_transpose(p, h, yn, q4)
                    nc.vector.tensor_copy(ypk_sb[c][:], ypkps[:])

            # ---------------- per-chunk pipeline ----------------
            # projections deferred toward late chunks whose attention windows
            # are ACT(exp)-bound and would otherwise leave the PE idle
            proj_sched = {2: [0], 3: [1, 2]}
            # chunk 0: only q/k of head-pair 0 (plus the first v pair, filling
            # x-arrival gaps) eagerly; the rest interleaves into attention(0)
            # so its exponentials overlap the projections
            qk_half(0, 0, 0)
            v_pair(0, 0)
            qk_half(0, 0, 1)
            for c in range(NCH):
                if c + 1 < NCH:
                    load_xc(c + 1)
                thunks = []
                if c == 0:
                    thunks += [
                        lambda: v_pair(0, 2),
                        lambda: qk_half(0, 1, 0),
                        lambda: qk_half(0, 1, 1),
                    ]
                if c + 1 < NCH:
                    if c + 1 == NCH - 1:
                        # the last chunk's second v pair is deferred into the
                        # final (ACT-bound) window where the PE has slack
                        thunks += [
                            lambda cc=c + 1: v_pair(cc, 0),
                            lambda cc=c + 1, p=0: qk_half(cc, p, 0),
                            lambda cc=c + 1, p=0: qk_half(cc, p, 1),
                            lambda cc=c + 1, p=1: qk_half(cc, p, 0),
                            lambda cc=c + 1, p=1: qk_half(cc, p, 1),
                        ]
                    else:
                        thunks += qkv_thunks(c + 1)
                if c == NCH - 1:
                    thunks = [lambda cc=c: v_pair(cc, 2)] + thunks
                for pc in proj_sched.get(c, []):
                    thunks += proj_thunks(pc)
                attention(c, thunks, last=(c == NCH - 1))
            for k, u in enumerate(range(NFT)):
                proj_u(NCH - 1, u, eng=("v", "s")[k % 2])

    nc.compile()
    return nc


def _get_nc():
    if "nc" not in _nc_cache:
        _nc_cache["nc"] = build_nc()
    return _nc_cache["nc"]


def _in_maps(x, W_attn, W_proj):
    bf = ml_dtypes.bfloat16
    tri = np.triu(np.ones((128, 128), np.float32)).astype(bf)
    ident = np.eye(128, dtype=np.float32).astype(bf)
    maps = []
    for core in range(NCORES):
        b, g = core // NG, core % NG
        lo = g * GC
        maps.append(
            {
                "xT": np.ascontiguousarray(x[b].T).astype(bf),
                "wq": np.ascontiguousarray(W_attn[:, lo : lo + GC]).astype(bf),
                "wk": np.ascontiguousarray(W_attn[:, C + lo : C + lo + GC]).astype(bf),
                "wv": np.ascontiguousarray(W_attn[:, 2 * C + lo : 2 * C + lo + GC]).astype(bf),
                "wpr": np.ascontiguousarray(W_proj[lo : lo + GC, :]).astype(bf),
                "tri": tri,
                "ident": ident,
            }
        )
    return maps


def kernel(x, W_attn, W_proj, **run_kwargs):
    x = np.asarray(x, np.float32)
    W_attn = np.asarray(W_attn, np.float32)
    W_proj = np.asarray(W_proj, np.float32)
    nc = _get_nc()
    res = run_bass_kernel_spmd(
        nc, _in_maps(x, W_attn, W_proj), core_ids=list(range(NCORES)), **run_kwargs
    )
    out = np.empty((B, T, C), np.float32)
    for b in range(B):
        acc = res.results[NG * b]["outP"].copy()
        for g in range(1, NG):
            acc += res.results[NG * b + g]["outP"]
        out[b] = acc.T
    if run_kwargs:
        kernel.last_result = res
    return out


# revision 32
# speedup vs baseline: 1.3075x; 1.0291x over previous
"""Causal self-attention (B=2, T=2048, C=1024, 16 heads) on 8 trn2 cores.

Sharding: 2 batches x 4 head-groups (4 heads each per core). Per core,
t-chunk-major pipeline in bf16 (fp32 psum accumulation):
  - stream x columns per 512-wide chunk (one DMA per chunk), project q/k into
    a shared [128, 2, T] bf16 tile per head-pair and v into a [128, 16, 4, 65]
    bf16 tile with a ones-column for the softmax denominator;
  - scores stay transposed [tk, tq] in a dedicated psum ring, exp on ACT into
    bf16 es tiles, causal diagonal masked via a tri multiply on DVE; the
    j-loop is software-pipelined (scores j+1 issued before av j);
  - AV is recast as out[tq, d+1] psum with the es 128x128 column block as the
    matmul stationary operand and v (65 cols) moving, so each matmul streams
    only 65 rows; the denominator lands per-partition and is normalized with
    reciprocal + broadcast multiply on DVE;
  - normalized y blocks are PE-transposed into a packed [128, 2, 512] bf16
    psum tile at chunk end (replacing pack DMAs), copied once to SBUF, and
    consumed by the output projection; qkv of chunk c+1 and the projection of
    chunk c-1 are emitted as fine-grained thunks interleaved into the
    attention j-loop so the PE always has independent work while ACT computes
    exponentials; projection partials are copied psum->sbuf on GPSIMD and
    DMAed out; the host sums the 4 per-core partials per batch.
"""
import numpy as np
import ml_dtypes

import concourse.bacc as bacc
import concourse.mybir as mybir
import concourse.tile as tile
from concourse.bass_utils import run_bass_kernel_spmd

F32 = mybir.dt.float32
BF16 = mybir.dt.bfloat16
FP8 = mybir.dt.float8e4
DR = mybir.MatmulPerfMode.DoubleRow
EXP = mybir.ActivationFunctionType.Exp

B, T, C = 2, 2048, 1024
NH, HD = 16, 64
NCORES = 8
NG = 4            # head groups (tensor-parallel within a batch)
GC = 256          # features per group (4 heads * 64)
NFT = C // 128    # 8 feature tiles
NTT = T // 128    # 16 t tiles
NCH = T // 512    # 4 tq chunks

_nc_cache = {}


def build_nc():
    nc = bacc.Bacc("TRN2", target_bir_lowering=False, debug=False, num_devices=NCORES)
    xT = nc.dram_tensor("xT", [C, T], BF16, kind="ExternalInput")
    wq = nc.dram_tensor("wq", [C, GC], BF16, kind="ExternalInput")
    wk = nc.dram_tensor("wk", [C, GC], BF16, kind="ExternalInput")
    wv = nc.dram_tensor("wv", [C, GC], BF16, kind="ExternalInput")
    wpr = nc.dram_tensor("wpr", [GC, C], BF16, kind="ExternalInput")
    tri = nc.dram_tensor("tri", [128, 128], BF16, kind="ExternalInput")
    ident = nc.dram_tensor("ident", [128, 128], BF16, kind="ExternalInput")
    outP = nc.dram_tensor("outP", [C, T], F32, kind="ExternalOutput")

    with tile.TileContext(nc) as tc:
        with (
            tc.tile_pool(name="consts", bufs=1) as consts,
            tc.tile_pool(name="xp", bufs=1) as xp,
            tc.tile_pool(name="wp", bufs=1) as wp,
            tc.tile_pool(name="qk", bufs=1) as qkp,
            tc.tile_pool(name="vbp", bufs=1) as vbp,
            tc.tile_pool(name="work", bufs=1) as work,
            tc.tile_pool(name="psum", bufs=1, space="PSUM") as ps,
        ):
            # weight tiles: partition = contraction (c) dim
            wqt = wp.tile([128, NFT, GC], BF16, name="wqt")
            wkt = wp.tile([128, NFT, GC], BF16, name="wkt")
            wvt = wp.tile([128, NFT, GC], BF16, name="wvt")
            wprt = wp.tile([128, 2, C], BF16, name="wprt")

            wq_r = wq[:].rearrange("(i p) f -> p i f", p=128)
            wk_r = wk[:].rearrange("(i p) f -> p i f", p=128)
            wv_r = wv[:].rearrange("(i p) f -> p i f", p=128)
            xT_r = xT[:].rearrange("(i p) t -> p i t", p=128)

            xct = {}

            def load_xc(c, split=False):
                xc = xp.tile([128, NFT, 512], BF16, tag="xc", bufs=2, name=f"xc{c}")
                if split:
                    # interleave x / wq feature slices so the first q matmuls
                    # start as early as possible (finest at the front)
                    for lo, hi in ((0, 1), (1, 2), (2, 4), (4, 6), (6, 8)):
                        nc.sync.dma_start(
                            xc[:, lo:hi, :], xT_r[:, lo:hi, 512 * c : 512 * (c + 1)]
                        )
                        nc.sync.dma_start(wqt[:, lo:hi, :], wq_r[:, lo:hi, :])
                else:
                    nc.sync.dma_start(xc[:], xT_r[:, :, 512 * c : 512 * (c + 1)])
                xct[c] = xc

            load_xc(0, split=True)
            nc.sync.dma_start(wvt[:], wv_r)
            nc.sync.dma_start(wkt[:], wk_r)
            nc.sync.dma_start(wprt[:], wpr[:].rearrange("(f d) c -> d f c", d=128))
            tri_sb = consts.tile([128, 128], BF16, name="tri_sb")
            nc.sync.dma_start(tri_sb[:], tri[:])
            id_sb = consts.tile([128, 128], BF16, name="id_sb")
            nc.sync.dma_start(id_sb[:], ident[:])
            # softmax shift: exp(x*0.125 - 3) keeps es within fp8-e4m3 range;
            # numerator and ones-column denominator scale identically
            nbias = consts.tile([128, 1], F32, name="nbias")
            nc.vector.memset(nbias[:], -3.0)

            # persistent per-core tensors
            qkT = [qkp.tile([128, 2, T], BF16, name=f"qkT{p}") for p in range(2)]
            # v with ones column at index 64 per (ttile, head-slot); bf16 for
            # the diagonal av matmuls, fp8 for the paired DoubleRow ones
            VB = vbp.tile([128, NTT, 4, 65], BF16, name="VB")
            nc.vector.memset(VB[:, :, :, 64:65], 1.0)
            VB8 = vbp.tile([128, NTT, 4, 65], FP8, name="VB8")
            nc.vector.memset(VB8[:, :, :, 64:65], 1.0)
            # packed normalized y^T per chunk (one per chunk: projections are
            # deferred into late-chunk windows where PE otherwise idles)
            ypk_sb = [
                work.tile([128, 2, 512], BF16, tag="ypk", bufs=NCH, name=f"ypk{m}")
                for m in range(NCH)
            ]

            def qk_half(c, p, which):
                # one [128, 512] projection (q or k) for head-pair p, chunk c
                w = wqt if which == 0 else wkt
                qkps = ps.tile([128, 512], F32, tag="big", bufs=2, name=f"qk{c}{p}{which}")
                for i in range(NFT):
                    nc.tensor.matmul(
                        qkps[:],
                        w[:, i, 128 * p : 128 * (p + 1)],
                        xct[c][:, i, :],
                        start=(i == 0),
                        stop=(i == NFT - 1),
                    )
                nc.vector.tensor_copy(qkT[p][:, which, 512 * c : 512 * (c + 1)], qkps[:])

            def v_pair(c, tl2):
                # v for two 128-t tiles (tl2, tl2+1) of chunk c.
                # one accumulation group for the whole psum bank: start only on
                # the very first write, stop on the last (psum start lazily
                # zeroes the entire 2KB bank region)
                vps = ps.tile([128, 512], F32, tag="big", bufs=2, name=f"v{c}{tl2}")
                for tl in (tl2, tl2 + 1):
                    for i in range(NFT):
                        nc.tensor.matmul(
                            vps[:, 256 * (tl - tl2) : 256 * (tl - tl2 + 1)],
                            xct[c][:, i, 128 * tl : 128 * (tl + 1)],
                            wvt[:, i, :],
                            start=(tl == tl2 and i == 0),
                            stop=(tl == tl2 + 1 and i == NFT - 1),
                        )
                nc.vector.tensor_copy(
                    VB[:, 4 * c + tl2 : 4 * c + tl2 + 2, :, 0:64],
                    vps[:].rearrange("p (s h d) -> p s h d", s=2, h=4),
                )
                nc.vector.tensor_copy(
                    VB8[:, 4 * c + tl2 : 4 * c + tl2 + 2, :, 0:64],
                    vps[:].rearrange("p (s h d) -> p s h d", s=2, h=4),
                )

            def proj_u(c, u, eng=None):
                # one 128-row output slice of the projection for chunk c
                yp = ypk_sb[c]
                opp = ps.tile([128, 512], F32, tag="big", bufs=2, name=f"opp{c}{u}")
                for p in range(2):
                    nc.tensor.matmul(
                        opp[:],
                        wprt[:, p, 128 * u : 128 * (u + 1)],
                        yp[:, p, :],
                        start=(p == 0),
                        stop=(p == 1),
                    )
                ob = work.tile([128, 512], F32, tag="osb", bufs=6, name=f"ob{c}{u}")
                if eng == "s":
                    nc.scalar.copy(ob[:], opp[:])
                else:
                    nc.vector.tensor_copy(ob[:], opp[:])
                nc.sync.dma_start(
                    outP[:]
                    .rearrange("(u d) t -> d u t", d=128)[
                        :, u, 512 * c : 512 * (c + 1)
                    ],
                    ob[:],
                )

            def qkv_thunks(c, skip_qk0=False):
                pairs = [(p, w) for p in range(2) for w in range(2)]
                if skip_qk0:
                    pairs = pairs[2:]
                return (
                    [lambda tl2=tl2: v_pair(c, tl2) for tl2 in (0, 2)]
                    + [lambda p=p, w=w: qk_half(c, p, w) for p, w in pairs]
                )

            def proj_thunks(c):
                return [lambda u=u: proj_u(c, u) for u in range(NFT)]

            def attention(c, thunks, last=False):
                nj = 4 * c + 4
                npts = 2 * nj
                emitted = 0
                pt = 0

                def maybe_thunk():
                    nonlocal emitted, pt
                    while emitted < len(thunks) and emitted <= pt * len(thunks) // npts:
                        thunks[emitted]()
                        emitted += 1
                    pt += 1

                if last:
                    ypkps = ps.tile(
                        [128, 2, 512], BF16, tag="big", bufs=2, name=f"ypkps{c}"
                    )
                def emit_transpose(pp, h, yn, q4):
                    # psum zero-region state is per partition: one accumulation
                    # group per 64-partition half of the ypkps bank (start on
                    # its first transpose, stop on its last); every write hits
                    # fresh (lazily zeroed) bytes so values pass through
                    nc.tensor.matmul(
                        ypkps[64 * h : 64 * (h + 1), pp, 128 * q4 : 128 * (q4 + 1)],
                        yn[:, q4, :],
                        id_sb[:],
                        start=(pp == 0 and q4 == 0),
                        stop=(pp == 1 and q4 == 3),
                        is_transpose=True,
                        # the interpreter's group-check bookkeeping mishandles
                        # 64-partition-offset psum APs; execution semantics
                        # (lazy zero + first-touch overwrite) remain checked
                        skip_group_check=True,
                    )

                yns = []
                for p in range(2):
                    yq = [
                        ps.tile([128, 260], F32, tag="yq", bufs=2, name=f"yq{c}{p}{h}")
                        for h in range(2)
                    ]

                    esp_cur = [None]

                    def scores_exp(j):
                        d = j - 4 * c
                        off = 128 * max(d, 0)
                        sps = ps.tile(
                            [128, 1024], F32, tag="sps", bufs=2, name=f"sps{c}{p}{j}"
                        )
                        for h in range(2):
                            nc.tensor.matmul(
                                sps[:, 512 * h + off : 512 * (h + 1)],
                                qkT[p][64 * h : 64 * (h + 1), 1, 128 * j : 128 * (j + 1)],
                                qkT[p][64 * h : 64 * (h + 1), 0, 512 * c + off : 512 * (c + 1)],
                                start=True,
                                stop=True,
                            )
                        if d < 0:
                            # full tile: exp straight to fp8, paired for the
                            # DoubleRow av matmuls
                            if j % 2 == 0:
                                esp_cur[0] = work.tile(
                                    [128, 2, 1024], FP8, tag="esp", bufs=2, name=f"esp{c}{p}{j}"
                                )
                            esp = esp_cur[0]
                            nc.scalar.activation(
                                esp[:, j % 2, :].rearrange("p (g n) -> p g n", g=2),
                                sps[:].rearrange("p (g n) -> p g n", g=2),
                                EXP,
                                scale=0.125,
                                bias=nbias[:],
                            )
                            return ("pair", j - 1, esp) if j % 2 == 1 else None
                        es = work.tile([128, 1024], BF16, tag="es", bufs=4, name=f"es{c}{p}{j}")
                        nc.scalar.activation(
                            es[:].rearrange("p (g n) -> p g n", g=2)[:, :, off:512],
                            sps[:].rearrange("p (g n) -> p g n", g=2)[:, :, off:512],
                            EXP,
                            scale=0.125,
                            bias=nbias[:],
                        )
                        for h in range(2):
                            nc.vector.tensor_mul(
                                es[:, 512 * h + off : 512 * h + off + 128],
                                es[:, 512 * h + off : 512 * h + off + 128],
                                tri_sb[:],
                            )
                        return ("single", j, es)

                    def av(item):
                        # one accumulation group per yq bank: start only on the
                        # very first write (its lazy zero covers the whole
                        # bank), stop on the very last
                        kind, j, t = item
                        for h in range(2):
                            hs = 2 * p + h
                            if kind == "pair":
                                for q4 in range(4):
                                    nc.tensor.matmul(
                                        yq[h][:, 65 * q4 : 65 * (q4 + 1)],
                                        t[:, :, 512 * h + 128 * q4 : 512 * h + 128 * (q4 + 1)],
                                        VB8[:, j : j + 2, hs, :],
                                        start=(not av_started[h]),
                                        stop=False,
                                        perf_mode=DR,
                                    )
                                    av_started[h] = True
                            else:
                                d = j - 4 * c
                                for q4 in range(max(d, 0), 4):
                                    nc.tensor.matmul(
                                        yq[h][:, 65 * q4 : 65 * (q4 + 1)],
                                        t[:, 512 * h + 128 * q4 : 512 * h + 128 * (q4 + 1)],
                                        VB[:, j, hs, :],
                                        start=(not av_started[h]),
                                        stop=(j == 4 * c + 3 and q4 == 3),
                                    )
                                    av_started[h] = True

                    av_started = [False, False]
                    prev = None
                    for j in range(nj):
                        ready = scores_exp(j)
                        maybe_thunk()
                        if prev is not None:
                            av(prev)
                        prev = ready
                        maybe_thunk()
                    av(prev)

                    for h in range(2):
                        rd = work.tile([128, 4], F32, tag="rd", bufs=2, name=f"rd{c}{p}{h}")
                        nc.vector.reciprocal_approx_fast(
                            rd[:], yq[h][:].rearrange("p (s n) -> p s n", s=4)[:, :, 64]
                        )
                        yn = work.tile(
                            [128, 4, 64], BF16, tag="yn", bufs=4, name=f"yn{c}{p}{h}"
                        )
                        nc.vector.tensor_mul(
                            yn[:],
                            yq[h][:].rearrange("p (s n) -> p s n", s=4)[:, :, 0:64],
                            rd[:, :, None].broadcast_to([128, 4, 64]),
                        )
                        if last:
                            # transpose + pack immediately so the final
                            # projection's tail is as short as possible
                            for q4 in range(4):
                                emit_transpose(p, h, yn, q4)
                        else:
                            yns.append((p, h, yn))

                    if last:
                        nc.vector.tensor_copy(
                            ypk_sb[c][:, p : p + 1, :], ypkps[:, p : p + 1, :]
                        )

                # drain any leftover thunks before the transpose burst
                while emitted < len(thunks):
                    thunks[emitted]()
                    emitted += 1

                if not last:
                    ypkps = ps.tile(
                        [128, 2, 512], BF16, tag="big", bufs=2, name=f"ypkps{c}"
                    )
                    for (p, h, yn) in yns:
                        for q4 in range(4):
                            emit_transpose(p, h, yn, q4)
                    nc.vector.tensor_copy(ypk_sb[c][:], ypkps[:])

            # ---------------- per-chunk pipeline ----------------
            # projections deferred toward late chunks whose attention windows
            # are ACT(exp)-bound and would otherwise leave the PE idle
            proj_sched = {2: [0], 3: [1, 2]}
            # chunk 0: only q/k of head-pair 0 (plus the first v pair, filling
            # x-arrival gaps) eagerly; the rest interleaves into attention(0)
            # so its exponentials overlap the projections
            qk_half(0, 0, 0)
            v_pair(0, 0)
            qk_half(0, 0, 1)
            for c in range(NCH):
                if c + 1 < NCH:
                    load_xc(c + 1)
                thunks = []
                if c == 0:
                    thunks += [
                        lambda: v_pair(0, 2),
                        lambda: qk_half(0, 1, 0),
                        lambda: qk_half(0, 1, 1),
                    ]
                if c + 1 < NCH:
                    if c + 1 == NCH - 1:
                        # the last chunk's second v pair is deferred into the
                        # final (ACT-bound) window where the PE has slack
                        thunks += [
                            lambda cc=c + 1: v_pair(cc, 0),
                            lambda cc=c + 1, p=0: qk_half(cc, p, 0),
                            lambda cc=c + 1, p=0: qk_half(cc, p, 1),
                            lambda cc=c + 1, p=1: qk_half(cc, p, 0),
                            lambda cc=c + 1, p=1: qk_half(cc, p, 1),
                        ]
                    else:
                        thunks += qkv_thunks(c + 1)
                if c == NCH - 1:
                    thunks = [lambda cc=c: v_pair(cc, 2)] + thunks
                for pc in proj_sched.get(c, []):
                    thunks += proj_thunks(pc)
                attention(c, thunks, last=(c == NCH - 1))
            for k, u in enumerate(range(NFT)):
                proj_u(NCH - 1, u, eng=("v", "s")[k % 2])

    nc.compile()
    return nc


def _get_nc():
    if "nc" not in _nc_cache:
        _nc_cache["nc"] = build_nc()
    return _nc_cache["nc"]


def _in_maps(x, W_attn, W_proj):
    bf = ml_dtypes.bfloat16
    tri = np.triu(np.ones((128, 128), np.float32)).astype(bf)
    ident = np.eye(128, dtype=np.float32).astype(bf)
    maps = []
    for core in range(NCORES):
        b, g = core // NG, core % NG
        lo = g * GC
        maps.append(
            {
                "xT": np.ascontiguousarray(x[b].T).astype(bf),
                "wq": np.ascontiguousarray(W_attn[:, lo : lo + GC]).astype(bf),
                "wk": np.ascontiguousarray(W_attn[:, C + lo : C + lo + GC]).astype(bf),
                "wv": np.ascontiguousarray(W_attn[:, 2 * C + lo : 2 * C + lo + GC]).astype(bf),
                "wpr": np.ascontiguousarray(W_proj[lo : lo + GC, :]).astype(bf),
                "tri": tri,
                "ident": ident,
            }
        )
    return maps


def kernel(x, W_attn, W_proj, **run_kwargs):
    x = np.asarray(x, np.float32)
    W_attn = np.asarray(W_attn, np.float32)
    W_proj = np.asarray(W_proj, np.float32)
    nc = _get_nc()
    res = run_bass_kernel_spmd(
        nc, _in_maps(x, W_attn, W_proj), core_ids=list(range(NCORES)), **run_kwargs
    )
    out = np.empty((B, T, C), np.float32)
    for b in range(B):
        acc = res.results[NG * b]["outP"].copy()
        for g in range(1, NG):
            acc += res.results[NG * b + g]["outP"]
        out[b] = acc.T
    if run_kwargs:
        kernel.last_result = res
    return out
